# revision 34
# baseline (speedup 1.0000x reference)
"""Trainium2 Bass kernel for nn_EnhancedDLinear (8-core SPMD, full I/O).

Mathematical reductions (vs the jax reference, verified numerically):

1. ``LayerNorm(1)`` output is the constant ``ln_b`` (size-1 normalization
   axis), so the detail branch (conv stack, adaptive softmax, [N,S,S]
   attention) is dead code; ``detail_pred`` is a weight-only constant
   row folded on the host.
2. The replicate-pad moving average (k=25) is a linear map folded into
   the first trend/seasonal MLP layers.
3. The channel-mean feeding the fusion MLP folds into its weights; the
   constant detail contribution folds into its bias.
4. Biases ride the matmuls via constant-one contraction rows (no
   broadcast-DMA bias tiles, no vector adds).
5. The fusion softmax normalizer folds into the final hidden Relu's
   per-partition ``scale`` operand, so unnormalized exponentials flow
   through the combine matmuls; its denominator comes from a ones-matmul
   partition reduction + DVE free-axis reduce.

All matmul operands are bf16 (tolerance 2e-2; measured ~2.5e-3), halving
DMA bytes vs fp32 and running the PE at 1 cycle/row at any moving width.
PSUM accumulation stays fp32.

Schedule notes (hard-won on real HW):
- DMA rows must be >=~1-2KB and partition counts multiples of 16 or the
  descriptors serialize onto one of the 16 DMA engines (~6x slower).
- wa splits in two u-aligned pieces so layer 1 starts on the first; on
  sync the tiny bias pack lands before the L2 weight (the relus gate on
  it), with the L2 weight right after for the sum-col matmuls.
- The [96,2] sum-cols are computed by dedicated skinny matmuls ahead of
  the wide L2 so the z-chain starts ~1us earlier; the wide L2 and the
  e-weighted staging muls overlap the chain.
- Output is column-split into two DMAs on separate queues.

Sharding: one batch per core (N = B*C, contiguous blocks of C=96), zero
collectives, tiny weights replicated.
"""

import numpy as np
import ml_dtypes

import concourse.bacc as bacc
import concourse.tile as tile
from concourse import mybir
from concourse.bass_utils import run_bass_kernel_spmd

B, S, C, P = 8, 336, 96, 96
HID = 168
MAIN_K = 25
N_CORES = 8
KC = 112          # contraction chunk (336 = 3*112)

_CACHE = {}


def _mavg_matrix(s, k):
    # mt = xc @ Mm for the replicate-padded moving average
    p = (k - 1) // 2
    m = np.zeros((s, s), np.float64)
    for j in range(s):
        for d in range(-p, p + 1):
            i = min(max(j + d, 0), s - 1)
            m[i, j] += 1.0 / k
    return m.astype(np.float32)


def _bf(a):
    return np.ascontiguousarray(a, np.float32).astype(ml_dtypes.bfloat16)


def _build_module():
    f32 = mybir.dt.float32
    bf16 = mybir.dt.bfloat16
    nc = bacc.Bacc("TRN2", target_bir_lowering=False, debug=False,
                   num_devices=N_CORES)

    xb = nc.dram_tensor("xb", [KC, 3 * C], bf16, kind="ExternalInput")
    wa = nc.dram_tensor("wa", [KC, 3 * S], bf16, kind="ExternalInput")
    wb = nc.dram_tensor("wb", [128, 592], bf16, kind="ExternalInput")
    # sp16a [96, 112]: fn1t (0:32) | fn1s (32:64) | fp1wT (64:112)
    sp16a = nc.dram_tensor("sp16a", [96, 112], bf16, kind="ExternalInput")
    # sp16b [49, 480]: fp2aug rows 0:49 (0:96) | Wk rows 0:33 (96+96k) |
    #                  dp_row row 0 (384:480)
    sp16b = nc.dram_tensor("sp16b", [49, 480], bf16, kind="ExternalInput")
    # spf f32 [112, 8]: b1 u-chunks (cols 0:3) | b1f rows 0:32 (col 3) |
    #                   fp1b rows 0:48 (col 4)
    spf = nc.dram_tensor("spf", [KC, 8], f32, kind="ExternalInput")
    y = nc.dram_tensor("y", [P, P], f32, kind="ExternalOutput")

    AF = mybir.ActivationFunctionType

    with tile.TileContext(nc) as tc:
        with (
            tc.tile_pool(name="wp", bufs=1) as wp,
            tc.tile_pool(name="hp", bufs=1) as hp,
            tc.tile_pool(name="pp", bufs=7, space="PSUM") as pp,
        ):
            xbs = wp.tile([KC, 3 * C], bf16, tag="xbs")
            was_a = wp.tile([KC, 2 * S], bf16, tag="was_a")
            was_b = wp.tile([KC, S], bf16, tag="was_b")
            was = [was_a[:, 0:S], was_a[:, S:2 * S], was_b]
            wbs = wp.tile([128, 592], bf16, tag="wbs")
            sp16a_s = wp.tile([96, 112], bf16, tag="sp16a")
            sp16b_s = wp.tile([49, 480], bf16, tag="sp16b")
            spf_s = wp.tile([KC, 8], f32, tag="spf")
            dpb = wp.tile([96, 96], bf16, tag="dpb")

            # wa first on scalar (2KB-row DMAs: small-row splits
            # transfer ~6x slower; SWDGE desc-gen is too slow for the big
            # tensors). Sync in need-order with small DMAs ahead of wbs so
            # wa/xb transfers stay uncontended.
            nc.scalar.dma_start(out=was_a, in_=wa[:, 0:2 * S])
            nc.scalar.dma_start(out=was_b, in_=wa[:, 2 * S:3 * S])
            nc.sync.dma_start(out=xbs, in_=xb[:, :])
            nc.sync.dma_start(out=spf_s, in_=spf[:, :])
            nc.sync.dma_start(out=wbs, in_=wb[:, :])
            nc.sync.dma_start(out=sp16a_s, in_=sp16a[:, :])
            nc.scalar.dma_start(out=sp16b_s, in_=sp16b[:, :])
            nc.scalar.dma_start(out=dpb,
                                in_=sp16b[0:1, 384:480].broadcast_to((96, 96)))

            # constant-one rows / tiles (off the DMA critical path)
            h1c = [hp.tile([KC + 1, 96], bf16, tag=f"h1c_{j}",
                           name=f"h1c_{j}") for j in range(3)]
            z1s = hp.tile([33, 1], bf16, tag="z1s")
            hs = hp.tile([49, 96], bf16, tag="hs")
            ones48 = hp.tile([96, 48], bf16, tag="ones48")
            # whole-tile memsets (partition offsets must be 32-aligned);
            # compute writes then overwrite the non-constant rows
            nc.vector.memset(h1c[2][:, :], 1.0)
            nc.vector.memset(z1s[:, :], 1.0)
            nc.vector.memset(hs[:, :], 1.0)
            nc.vector.memset(ones48[:, :], 1.0)

            # ---- layer 1: u-major so each psum closes early ----
            psu = [pp.tile([KC, 96], f32, tag="ps", name=f"psu{u}")
                   for u in range(3)]
            for u in range(3):
                for j in range(3):
                    nc.tensor.matmul(
                        psu[u], was[u][:, KC * j:KC * (j + 1)],
                        xbs[:, C * j:C * (j + 1)],
                        start=(j == 0), stop=(j == 2))
            # relus spread over scalar/vector so they finish ~in parallel
            nc.scalar.activation(h1c[0][0:KC, :], psu[0], AF.Relu,
                                 bias=spf_s[:, 0:1])
            nc.vector.tensor_scalar(h1c[1][0:KC, :], psu[1],
                                    spf_s[:, 1:2], 0.0,
                                    mybir.AluOpType.add, mybir.AluOpType.max)
            nc.scalar.activation(h1c[2][0:KC, :], psu[2], AF.Relu,
                                 bias=spf_s[:, 2:3])

            # ---- L2 sum-cols first (feeds the serial softmax chain) ----
            ps_sums = pp.tile([96, 2], f32, tag="ps", name="ps_sums")
            nc.tensor.matmul(ps_sums, h1c[0][0:KC, :], wbs[0:KC, 192:194],
                             start=True, stop=False)
            nc.tensor.matmul(ps_sums, h1c[1][0:KC, :], wbs[0:KC, 386:388],
                             start=False, stop=False)
            nc.tensor.matmul(ps_sums, h1c[2][0:KC + 1, :],
                             wbs[0:KC + 1, 580:582], start=False, stop=True)
            ts2 = hp.tile([96, 2], bf16, tag="ts2")
            nc.scalar.activation(ts2, ps_sums, AF.Copy)

            # ---- wide L2 [tp | sp], z1 slotted between chunks ----
            ps_l2 = pp.tile([96, 192], f32, tag="ps")
            ps_z1 = pp.tile([32, 1], f32, tag="ps", name="ps_z1")
            nc.tensor.matmul(ps_l2, h1c[0][0:KC, :], wbs[0:KC, 0:192],
                             start=True, stop=False)
            nc.tensor.matmul(ps_z1, sp16a_s[:, 0:32], ts2[:, 0:1],
                             start=True, stop=False)
            nc.tensor.matmul(ps_z1, sp16a_s[:, 32:64], ts2[:, 1:2],
                             start=False, stop=True)
            nc.tensor.matmul(ps_l2, h1c[1][0:KC, :], wbs[0:KC, 194:386],
                             start=False, stop=False)
            nc.tensor.matmul(ps_l2, h1c[2][0:KC + 1, :],
                             wbs[0:KC + 1, 388:580], start=False, stop=True)
            nc.vector.tensor_scalar(z1s[0:32, :], ps_z1,
                                    spf_s[0:32, 3:4], 0.0,
                                    mybir.AluOpType.add, mybir.AluOpType.max)
            at_s = hp.tile([96, 96], bf16, tag="at_s")
            nc.vector.tensor_copy(at_s, ps_l2[:, 0:96])
            asl_s = hp.tile([96, 96], bf16, tag="asl_s")
            nc.vector.tensor_copy(asl_s, ps_l2[:, 96:192])

            # z-cols [96c, 3k] directly (no row->col transposes)
            zc = pp.tile([96, 3], f32, tag="ps", name="zc")
            for k in range(3):
                nc.tensor.matmul(zc[:, k:k + 1],
                                 sp16b_s[0:33, 96 + 96 * k:192 + 96 * k],
                                 z1s, skip_group_check=True)
            ec = hp.tile([96, 3], f32, tag="ec")
            nc.scalar.activation(ec, zc, AF.Exp)
            ecb = hp.tile([96, 3], bf16, tag="ecb")
            nc.gpsimd.tensor_copy(ecb, ec)

            # e-weights fold into small [96,48] stationary muls
            wt = hp.tile([96, 48], bf16, tag="wt")
            nc.vector.tensor_scalar_mul(wt, sp16a_s[:, 64:112], ec[:, 0:1])
            ws = hp.tile([96, 48], bf16, tag="ws")
            nc.vector.tensor_scalar_mul(ws, sp16a_s[:, 64:112], ec[:, 1:2])
            wd = hp.tile([96, 48], bf16, tag="wd")
            nc.scalar.activation(wd, sp16a_s[:, 64:112], AF.Copy,
                                 scale=ec[:, 2:3])

            # denominator -> per-partition recip for the Relu scale
            den48q = pp.tile([48, 3], f32, tag="ps")
            nc.tensor.matmul(den48q, ones48, ecb, start=True, stop=True)
            dsum = hp.tile([48, 1], f32, tag="dsum")
            nc.vector.tensor_reduce(dsum, den48q, mybir.AxisListType.X,
                                    mybir.AluOpType.add)
            recip48 = hp.tile([48, 1], f32, tag="recip48")
            nc.vector.reciprocal(recip48, dsum)

            # ps_h = fp1w @ (e0*tp + e1*sp + e2*dp), unnormalized
            ps_h = pp.tile([48, 96], f32, tag="ps")
            nc.tensor.matmul(ps_h, wt, at_s, start=True, stop=False)
            nc.tensor.matmul(ps_h, wd, dpb, start=False, stop=False)
            nc.tensor.matmul(ps_h, ws, asl_s, start=False, stop=True)

            nc.scalar.activation(hs[0:48, :], ps_h, AF.Relu,
                                 bias=spf_s[0:48, 4:5], scale=recip48)

            ps_o = pp.tile([96, 96], f32, tag="ps")
            nc.tensor.matmul(ps_o[:, 0:48], hs, sp16b_s[:, 0:48],
                             start=True, stop=True, skip_group_check=True)
            nc.tensor.matmul(ps_o[:, 48:96], hs, sp16b_s[:, 48:96],
                             start=True, stop=True, skip_group_check=True)
            out_s = hp.tile([96, 96], f32, tag="out")
            nc.vector.tensor_copy(out_s[:, 0:48], ps_o[:, 0:48])
            nc.scalar.activation(out_s[:, 48:96], ps_o[:, 48:96], AF.Copy)
            nc.sync.dma_start(out=y[:, 0:48], in_=out_s[:, 0:48])
            nc.scalar.dma_start(out=y[:, 48:96], in_=out_s[:, 48:96])

    nc.compile()
    return nc


def _prep_weights(i):
    f = np.float32
    mm = _mavg_matrix(S, MAIN_K)
    w1 = np.empty((S, 2 * HID), f)
    w1[:, :HID] = mm @ i['lt1w'].T.astype(f)
    w1[:, HID:] = (np.eye(S, dtype=f) - mm) @ i['ls1w'].T.astype(f)
    # u-major: block u holds its three K-chunks contiguously
    wa = np.empty((KC, 3 * S), f)
    for u in range(3):
        for j in range(3):
            wa[:, S * u + KC * j:S * u + KC * (j + 1)] = \
                w1[KC * j:KC * (j + 1), KC * u:KC * (u + 1)]

    # constant detail_pred row (LayerNorm(1) output == ln_b exactly)
    xf = np.full((S,), f(i['ln_b'][0]), f)
    dp_row = (np.maximum(xf @ i['op1w'].T + i['op1b'], 0)
              @ i['op2w'].T + i['op2b']).astype(f)
    dpm = dp_row.mean(dtype=np.float32)
    # z1 bias: only fn1b + dp-mean term (lt2b/ls2b ride the L2 ones-row)
    b1f = (i['fn1b'] + dpm * i['fn1w'][:, 2 * C:].sum(1)).astype(f)

    lt2wt = np.ascontiguousarray(i['lt2w'].T, f)
    ls2wt = np.ascontiguousarray(i['ls2w'].T, f)
    # [337, 194] = [tp 0:96 | sp 96:192 | tps 192 | sps 193]; row 336 = bias
    w2full = np.zeros((S + 1, 194), f)
    w2full[0:HID, 0:96] = lt2wt
    w2full[0:HID, 192] = lt2wt.sum(1)
    w2full[HID:S, 96:192] = ls2wt
    w2full[HID:S, 193] = ls2wt.sum(1)
    w2full[S, 0:96] = i['lt2b']
    w2full[S, 192] = i['lt2b'].sum(dtype=np.float64)
    w2full[S, 96:192] = i['ls2b']
    w2full[S, 193] = i['ls2b'].sum(dtype=np.float64)
    wb = np.zeros((128, 592), f)
    for j in range(2):
        wb[0:KC, 194 * j:194 * (j + 1)] = w2full[KC * j:KC * (j + 1)]
    wb[0:KC, 388:582] = w2full[2 * KC:S]
    wb[KC, 388:582] = w2full[S]

    sp16a = np.zeros((96, 112), f)
    sp16a[:, 0:32] = i['fn1w'][:, 0:C].T / C
    sp16a[:, 32:64] = i['fn1w'][:, C:2 * C].T / C
    sp16a[:, 64:112] = i['fp1w'].T

    fn2T = np.ascontiguousarray(i['fn2w'].T, f)
    fn2b = i['fn2b'].astype(f)
    sp16b = np.zeros((49, 480), f)
    sp16b[0:48, 0:96] = i['fp2w'].T
    sp16b[48, 0:96] = i['fp2b']
    for k in range(3):
        sp16b[0:32, 96 + 96 * k:192 + 96 * k] = fn2T[:, 96 * k:96 * (k + 1)]
        sp16b[32, 96 + 96 * k:192 + 96 * k] = fn2b[96 * k:96 * (k + 1)]
    sp16b[0, 384:480] = dp_row

    b1 = np.concatenate([i['lt1b'], i['ls1b']]).astype(f)
    spf = np.zeros((KC, 8), f)
    for j in range(3):
        spf[:, j] = b1[KC * j:KC * (j + 1)]
    spf[0:32, 3] = b1f
    spf[0:48, 4] = i['fp1b']

    return dict(wa=_bf(wa), wb=_bf(wb), sp16a=_bf(sp16a),
                sp16b=_bf(sp16b), spf=spf)


def make_in_maps(inputs):
    shared = _prep_weights(inputs)
    x = np.asarray(inputs['x'], np.float32)
    in_maps = []
    for b in range(N_CORES):
        xbp = np.empty((KC, 3 * C), np.float32)
        for j in range(3):
            xbp[:, C * j:C * (j + 1)] = x[b, KC * j:KC * (j + 1), :]
        in_maps.append(dict(shared, xb=_bf(xbp)))
    return in_maps


def kernel(**inputs):
    if "nc" not in _CACHE:
        _CACHE["nc"] = _build_module()
    res = run_bass_kernel_spmd(_CACHE["nc"], make_in_maps(inputs),
                               core_ids=list(range(N_CORES)))
    return np.stack([res.results[b]["y"] for b in range(N_CORES)], 0)
